# revision 1
# baseline (speedup 1.0000x reference)
"""DenseAqt (int8 fake-quant dense layer) Trainium2 Bass kernel.

Full-input contract: kernel(x, kernel, bias) -> y, with x [65536, 512] f32,
kernel [512, 512] f32, bias [512] f32, y [65536, 512] f32.

Strategy (8 NeuronCores, data-parallel over rows; everything on-device):
  - per core: 8192 rows of x; weights/bias replicated.
  - w-prep (once): w_bound = colmax|w| (abs_max tree + PE transpose + free-axis
    reduce), w_scale = 127/w_bound, w_q = clip(rne(w*w_scale)) as EXACT
    integers in bf16; inv = 1/(a_scale*w_scale) broadcast [128, F]; bias
    folded as b2 = bias*a_scale*w_scale (bf16) added via K=1 matmuls into
    PSUM before the main accumulation.
  - x pipeline per [128, 2048] mega-tile: DVE (mult a_scale, max -127),
    DVE (min 127, add 1.5*2^23) [magic-constant RNE round], ACT copy with
    bias=-magic -> bf16 integers; PE transpose-matmuls (bf16, vs identity)
    to put K on partitions; ACT copy PSUM->SBUF.
  - main matmuls in bf16: exact (|x_q|<=127, |w_q|<=127, K=512 -> sums < 2^23
    are exactly representable in fp32 PSUM).
  - dequant fused into the mandatory PSUM->SBUF move: one DVE tensor_tensor
    multiply by inv (bias already inside via the K=1 matmuls).
"""

import numpy as np
import ml_dtypes

import concourse.bass as bass
import concourse.mybir as mybir
from concourse import tile
from concourse.bass_utils import run_bass_kernel_spmd

# ---- problem constants (hardcoded per contract) ----
N_ROWS = 65536
K_DIM = 512
F_DIM = 512
N_CORES = 8
ROWS_PER_CORE = N_ROWS // N_CORES        # 8192
MEGA_ROWS = 512                          # rows per mega-tile ([128, 2048])
N_MEGA = ROWS_PER_CORE // MEGA_ROWS      # 16
BLOCKS_PER_MEGA = MEGA_ROWS // 128       # 4 row-blocks per mega-tile
P = 128

CLIP = 127.0
A_SCALE = float(np.float32(127.0 / 3.0))
EPS = 1e-7
MAGIC = 12582912.0                       # 1.5 * 2**23: fp32 RNE rounding shift

F32 = mybir.dt.float32
BF16 = mybir.dt.bfloat16


# ---------------------------------------------------------------------------
# walrus workaround: this compiler build rejects >=2 sync waits per
# instruction; split extras onto same-engine NoOps placed just before.
_wsplit_ctr = [0]


def _split_waits(nc):
    for f in nc.m.functions:
        for b in f.blocks:
            insts = b.instructions
            out = []
            changed = False
            for inst in insts:
                si = inst.sync_info
                if si is not None and len(si.on_wait) > 1:
                    waits = list(si.on_wait)
                    for w in waits[:-1]:
                        _wsplit_ctr[0] += 1
                        out.append(
                            mybir.InstNoOp(
                                name=f"WSPLIT-{_wsplit_ctr[0]}",
                                engine=inst.engine,
                                bass_nofuse=True,
                                sync_info=mybir.SyncInfo(on_wait=[w], on_update=[]),
                            )
                        )
                    si.on_wait = [waits[-1]]
                    changed = True
                out.append(inst)
            if changed:
                try:
                    b.instructions[:] = out
                except TypeError:
                    b.instructions = out


# ---------------------------------------------------------------------------
def build_bass(rows_per_core=ROWS_PER_CORE, split_waits=True, repeat=1, bufs=None):
    n_mega = rows_per_core // MEGA_ROWS
    bufs = dict(dict(xload=5, xtmp=3, xqp=3, xqtp=4, ysb=3, tp_ps=3, y_ps=2), **(bufs or {}))
    nc = bass.Bass("TRN2", target_bir_lowering=False, debug=False, num_devices=1)

    xs = nc.dram_tensor("xs", [rows_per_core, K_DIM], F32, kind="ExternalInput").ap()
    w = nc.dram_tensor("w", [K_DIM, F_DIM], F32, kind="ExternalInput").ap()
    bias_in = nc.dram_tensor("bias_in", [1, F_DIM], F32, kind="ExternalInput").ap()
    ident_bf = nc.dram_tensor("ident_bf", [P, P], BF16, kind="ExternalInput").ap()
    ident_f32 = nc.dram_tensor("ident_f32", [P, P], F32, kind="ExternalInput").ap()
    ones_bf = nc.dram_tensor("ones_bf", [2, P], BF16, kind="ExternalInput").ap()
    ones_f32 = nc.dram_tensor("ones_f32", [1, P], F32, kind="ExternalInput").ap()
    ys = nc.dram_tensor("ys", [rows_per_core, F_DIM], F32, kind="ExternalOutput").ap()

    AL = mybir.AluOpType
    AF = mybir.ActivationFunctionType

    with tile.TileContext(nc) as tc:
        with (
            tc.tile_pool(name="pers", bufs=1) as pers,
            tc.tile_pool(name="xload", bufs=bufs["xload"]) as xload,
            tc.tile_pool(name="xtmp", bufs=bufs["xtmp"]) as xtmp,
            tc.tile_pool(name="xqp", bufs=bufs["xqp"]) as xqp,
            tc.tile_pool(name="xqtp", bufs=bufs["xqtp"]) as xqtp,
            tc.tile_pool(name="ysb", bufs=bufs["ysb"]) as ysbp,
            tc.tile_pool(name="tp_ps", bufs=bufs["tp_ps"], space="PSUM") as tp_ps,
            tc.tile_pool(name="y_ps", bufs=bufs["y_ps"], space="PSUM") as y_ps_pool,
        ):
            # ---------------- constants ----------------
            identb = pers.tile([P, P], BF16, tag="identb")
            nc.sync.dma_start(out=identb[:], in_=ident_bf[:])
            identf = pers.tile([P, P], F32, tag="identf")
            nc.sync.dma_start(out=identf[:], in_=ident_f32[:])
            onesb = pers.tile([2, P], BF16, tag="onesb")
            nc.sync.dma_start(out=onesb[:], in_=ones_bf[:])
            onesf = pers.tile([1, P], F32, tag="onesf")
            nc.sync.dma_start(out=onesf[:], in_=ones_f32[:])

            # ---------------- w-prep ----------------
            wf = []
            for c in range(4):
                t = pers.tile([P, F_DIM], F32, tag=f"wf{c}")
                nc.sync.dma_start(out=t[:], in_=w[128 * c : 128 * (c + 1), :])
                wf.append(t)
            bias_sb = pers.tile([1, F_DIM], F32, tag="bias_row")
            nc.sync.dma_start(out=bias_sb[:], in_=bias_in[:])

            # abs-max over the K partition-chunks (elementwise): |w| = max(w, -w)
            wabs = []
            for c in range(4):
                ng = pers.tile([P, F_DIM], F32, tag=f"wneg{c}")
                nc.vector.tensor_scalar(
                    out=ng[:], in0=wf[c][:], scalar1=-1.0, scalar2=None, op0=AL.mult
                )
                ab = pers.tile([P, F_DIM], F32, tag=f"wabs{c}")
                nc.vector.tensor_tensor(out=ab[:], in0=wf[c][:], in1=ng[:], op=AL.max)
                wabs.append(ab)
            ta = pers.tile([P, F_DIM], F32, tag="ta")
            nc.vector.tensor_tensor(out=ta[:], in0=wabs[0][:], in1=wabs[1][:], op=AL.max)
            tb = pers.tile([P, F_DIM], F32, tag="tb")
            nc.vector.tensor_tensor(out=tb[:], in0=wabs[2][:], in1=wabs[3][:], op=AL.max)
            tm = pers.tile([P, F_DIM], F32, tag="tm")
            nc.vector.tensor_tensor(out=tm[:], in0=ta[:], in1=tb[:], op=AL.max)

            # transpose tm chunks to put F on partitions; reduce over K-residue
            tmT = y_ps_pool.tile([P, 1024], F32, tag="y")  # reuse y psum slots
            for c in range(4):
                nc.tensor.transpose(
                    tmT[:, 128 * c : 128 * (c + 1)], tm[:, 128 * c : 128 * (c + 1)], identf[:]
                )
            wbT = pers.tile([P, 4], F32, tag="wbT")
            for c in range(4):
                nc.vector.reduce_max(
                    out=wbT[:, c : c + 1],
                    in_=tmT[:, 128 * c : 128 * (c + 1)],
                    axis=mybir.AxisListType.X,
                )
            # w_scale^T = 127 * recip(max(wbound, EPS))   [128, 4] (F on partitions)
            wb2 = pers.tile([P, 4], F32, tag="wb2")
            nc.vector.tensor_scalar(
                out=wb2[:], in0=wbT[:], scalar1=EPS, scalar2=None, op0=AL.max
            )
            wrT = pers.tile([P, 4], F32, tag="wrT")
            nc.vector.reciprocal(out=wrT[:], in_=wb2[:])
            wsT = pers.tile([P, 4], F32, tag="wsT")
            nc.vector.tensor_scalar(
                out=wsT[:], in0=wrT[:], scalar1=CLIP, scalar2=None, op0=AL.mult
            )
            # row layout [1, 512]: transpose each [128,1] column of wsT into
            # adjacent [1,128] spans of one PSUM row, then one copy out.
            wsq_ps = y_ps_pool.tile([P, 1024], F32, tag="y")
            for q in range(4):
                nc.tensor.transpose(
                    wsq_ps[:1, 128 * q : 128 * q + 128], wsT[:, q : q + 1], identf[:]
                )
            ws_all = pers.tile([1, F_DIM], F32, tag="ws_all")
            nc.vector.tensor_copy(out=ws_all[:], in_=wsq_ps[:1, 0:512])
            d_all = pers.tile([1, F_DIM], F32, tag="d_all")
            nc.vector.tensor_scalar(
                out=d_all[:], in0=ws_all[:], scalar1=A_SCALE, scalar2=None, op0=AL.mult
            )
            inv_all = pers.tile([1, F_DIM], F32, tag="inv_all")
            nc.vector.reciprocal(out=inv_all[:], in_=d_all[:])
            # b2 = bias * d, split into bf16 hi+lo so the K=2 bias matmul adds
            # it to ~2^-18 relative accuracy (one matmul, same cost as K=1).
            b2f = pers.tile([1, F_DIM], F32, tag="b2f")
            nc.vector.tensor_tensor(out=b2f[:], in0=bias_sb[:], in1=d_all[:], op=AL.mult)
            b2hi = pers.tile([1, F_DIM], BF16, tag="b2hi")
            nc.vector.tensor_copy(out=b2hi[:], in_=b2f[:])
            b2hi32 = pers.tile([1, F_DIM], F32, tag="b2hi32")
            nc.vector.tensor_copy(out=b2hi32[:], in_=b2hi[:])
            b2lo32 = pers.tile([1, F_DIM], F32, tag="b2lo32")
            nc.vector.tensor_tensor(
                out=b2lo32[:], in0=b2f[:], in1=b2hi32[:], op=AL.subtract
            )
            b2lo = pers.tile([1, F_DIM], BF16, tag="b2lo")
            nc.vector.tensor_copy(out=b2lo[:], in_=b2lo32[:])
            b2pair = pers.tile([2, F_DIM], BF16, tag="b2pair")
            nc.sync.dma_start(out=b2pair[0:1, :], in_=b2hi[:])
            nc.sync.dma_start(out=b2pair[1:2, :], in_=b2lo[:])

            # broadcast w_scale row -> [128, 512] via ones-column matmul (fp32)
            wsb_ps = y_ps_pool.tile([P, 1024], F32, tag="y")
            nc.tensor.matmul(
                wsb_ps[:, 0:512], onesf[:], ws_all[:], start=True, stop=True
            )
            wsb = pers.tile([P, F_DIM], F32, tag="wsb")
            nc.vector.tensor_copy(out=wsb[:], in_=wsb_ps[:, 0:512])

            # broadcast inv row -> [128, 1024] (two copies side by side)
            invb_ps = y_ps_pool.tile([P, 1024], F32, tag="y")
            for h in range(2):
                nc.tensor.matmul(
                    invb_ps[:, 512 * h : 512 * (h + 1)],
                    onesf[:],
                    inv_all[:],
                    start=True,
                    stop=True,
                )
            invb = pers.tile([P, 1024], F32, tag="invb")
            nc.vector.tensor_copy(out=invb[:], in_=invb_ps[:])

            # quantize w: w_q = clip(rne(w * w_scale), +-127) in bf16 (exact ints)
            wq = []
            for c in range(4):
                g = pers.tile([P, F_DIM], F32, tag=f"wg{c}")
                nc.vector.tensor_tensor(out=g[:], in0=wf[c][:], in1=wsb[:], op=AL.mult)
                g2 = pers.tile([P, F_DIM], F32, tag=f"wg2{c}")
                nc.vector.tensor_scalar(
                    out=g2[:], in0=g[:], scalar1=-CLIP, scalar2=CLIP,
                    op0=AL.max, op1=AL.min,
                )
                q = pers.tile([P, F_DIM], BF16, tag=f"wq{c}")
                nc.vector.tensor_scalar(
                    out=q[:], in0=g2[:], scalar1=MAGIC, scalar2=MAGIC,
                    op0=AL.add, op1=AL.subtract,
                )
                wq.append(q)

            # ---------------- main loop ----------------
            for m in [mm for _ in range(repeat) for mm in range(n_mega)]:
                r0 = m * MEGA_ROWS
                xf = xload.tile([P, 2048], F32, tag="xf")
                nc.sync.dma_start(
                    out=xf[:].rearrange("p (b k) -> p b k", b=BLOCKS_PER_MEGA),
                    in_=xs[r0 : r0 + MEGA_ROWS, :].rearrange("(b p) k -> p b k", p=P),
                )
                t1 = xtmp.tile([P, 2048], F32, tag="t1")
                nc.vector.tensor_scalar(
                    out=t1[:], in0=xf[:], scalar1=A_SCALE, scalar2=-CLIP,
                    op0=AL.mult, op1=AL.max,
                )
                t2 = xtmp.tile([P, 2048], F32, tag="t2")
                nc.gpsimd.tensor_scalar(
                    out=t2[:], in0=t1[:], scalar1=CLIP, scalar2=MAGIC,
                    op0=AL.min, op1=AL.add,
                )
                xq = xqp.tile([P, 2048], BF16, tag="xq")
                nc.scalar.activation(out=xq[:], in_=t2[:], func=AF.Copy, bias=-MAGIC)

                y_sb = ysbp.tile([P, 2048], F32, tag="ysb")
                for h in range(2):  # halves: 2 row-blocks each
                    xqt_ps = tp_ps.tile([P, 1024], BF16, tag="xqt")
                    for bl in range(2):  # local block within half
                        b = 2 * h + bl
                        for c in range(4):
                            nc.tensor.transpose(
                                xqt_ps[:, 512 * bl + 128 * c : 512 * bl + 128 * (c + 1)],
                                xq[:, 512 * b + 128 * c : 512 * b + 128 * (c + 1)],
                                identb[:],
                            )
                    xqt = xqtp.tile([P, 1024], BF16, tag="xqt_sb")
                    nc.scalar.copy(xqt[:], xqt_ps[:])

                    y_ps = y_ps_pool.tile([P, 1024], F32, tag="y")
                    for bl in range(2):
                        nc.tensor.matmul(
                            y_ps[:, 512 * bl : 512 * (bl + 1)],
                            onesb[:],
                            b2pair[:],
                            start=True,
                            stop=False,
                        )
                        for c in range(4):
                            nc.tensor.matmul(
                                y_ps[:, 512 * bl : 512 * (bl + 1)],
                                xqt[:, 512 * bl + 128 * c : 512 * bl + 128 * (c + 1)],
                                wq[c][:],
                                start=False,
                                stop=(c == 3),
                            )
                    nc.vector.tensor_tensor(
                        out=y_sb[:, 1024 * h : 1024 * (h + 1)],
                        in0=y_ps[:], in1=invb[:], op=AL.mult,
                    )
                    if m == n_mega - 1:
                        # tail: store each half as soon as it is ready so the
                        # final DMA overlaps the last compute instead of
                        # waiting for the whole mega-tile.
                        nc.sync.dma_start(
                            out=ys[r0 + 256 * h : r0 + 256 * (h + 1), :].rearrange(
                                "(b p) f -> p b f", p=P
                            ),
                            in_=y_sb[:, 1024 * h : 1024 * (h + 1)].rearrange(
                                "p (b f) -> p b f", b=2
                            ),
                        )
                if m != n_mega - 1:
                    nc.sync.dma_start(
                        out=ys[r0 : r0 + MEGA_ROWS, :].rearrange("(b p) f -> p b f", p=P),
                        in_=y_sb[:].rearrange("p (b f) -> p b f", b=BLOCKS_PER_MEGA),
                    )

    if split_waits:
        _split_waits(nc)
    return nc


_NC_CACHE = None


def kernel(x, kernel, bias):
    global _NC_CACHE
    if _NC_CACHE is None:
        _NC_CACHE = build_bass()
    nc = _NC_CACHE

    x = np.ascontiguousarray(x, dtype=np.float32)
    w = np.ascontiguousarray(kernel, dtype=np.float32)
    b = np.ascontiguousarray(bias, dtype=np.float32)

    ident_bf = np.eye(P, dtype=np.float32).astype(ml_dtypes.bfloat16)
    ident_f32 = np.eye(P, dtype=np.float32)
    ones_bf = np.ones((2, P), dtype=np.float32).astype(ml_dtypes.bfloat16)
    ones_f32 = np.ones((1, P), dtype=np.float32)
    bias_row = b.reshape(1, F_DIM)

    in_maps = []
    for i in range(N_CORES):
        in_maps.append(
            {
                "xs": x[i * ROWS_PER_CORE : (i + 1) * ROWS_PER_CORE],
                "w": w,
                "bias_in": bias_row,
                "ident_bf": ident_bf,
                "ident_f32": ident_f32,
                "ones_bf": ones_bf,
                "ones_f32": ones_f32,
            }
        )
    res = run_bass_kernel_spmd(nc, in_maps, core_ids=list(range(N_CORES)))
    return np.concatenate([res.results[i]["ys"] for i in range(N_CORES)], axis=0)

